# revision 14
# baseline (speedup 1.0000x reference)
"""Trainium2 Bass kernel for BasicCNN+LSTM (conv3x3+ReLU+GAP -> custom LSTM scan).

Self-contained: hardcodes shapes/sharding. Data-parallel over batch B=8 across
8 NeuronCores; each core processes one batch element end-to-end, the host
gathers the 8 [32] results.

v2 design (vs. baseline's 2-pixel bf16 scheme):
  - Conv as fp8(e4m3) DoubleRow matmuls: 8-pixel vertical blocks, K=90 taps
    (3c x 3dx x 10 window rows) split into 2 k-tiles of 45, M=128 = 16
    filters x 8 pixel positions (f-major), 3 filter groups. Moving columns
    per frame: 3 x 1568 (25% less than baseline, at 2x fp8 column rate),
    robust to the PE HAM clock gate (cold 1.2GHz still fits).
  - ReLU(+bias)+GAP as ONE instruction per (frame, filter-group) over a
    strided [128, 4, 392] view of a 4-bank PSUM tile, alternating
    Scalar ACT / Vector tensor_scalar, each with a bf16 accum_out column.
  - Scan step: 4 tiny accumulating matmuls (3x K=128 reading the gsum
    columns directly - the stationary replicates weights across the 8 pixel
    positions so the cross-partition fold is absorbed - plus K=32 for the
    cell part), one sigmoid ACT with per-partition scale [1,1,2] and bias
    [b1,b2,2*b3] (tanh(a)=2*sigmoid(2a)-1), and 6 tiny ALU ops spread over
    GpSimd/Vector. Reference's state-order swap bug kept: the z "hidden"
    input is the previous cell, and gate1 multiplies the previous hidden.
"""
import sys
if '/opt/trn_rl_repo' not in sys.path:
    sys.path.insert(0, '/opt/trn_rl_repo')

import numpy as np
import ml_dtypes

import concourse.bass as bass
import concourse.mybir as mybir
import concourse.tile as tile
from concourse.vector_clock import ScopedClock
from concourse.bass_utils import run_bass_kernel_spmd

# ---------------------------------------------------------------- constants
B, T, H, W, C, F, U = 8, 24, 112, 112, 3, 48, 32
KP = 90            # taps: 3c x 3dx x 10 window rows
JA = 14            # vertical 8-pixel blocks per frame (112/8)
NCOL = JA * W      # 1568 moving columns per frame per filter group
NCH = 392          # columns per matmul chunk (4 chunks, bank-aligned @512)
M = 128            # out partitions = 16 filters x 8 pixel positions
SBK = 2            # banks of each psum tile handled by ScalarE (rest: DVE)
HWN = float(H * W)

FP32 = mybir.dt.float32
BF16 = mybir.dt.bfloat16
FP8 = mybir.dt.float8e4

LAST_RESULTS = None  # BassKernelResults of the most recent run (for test.py)

# ------------------------------------------------- TileContext drain patch
# The container's walrus rejects >1 semaphore wait per instruction; Tile's
# kernel-tail drain aggregates all end-of-kernel waits onto one Drain.
# Spread them across single-wait NOPs on the sync engine instead.
def _patched_drain_and_barrier(self, tick_clock, wait_clock):
    nc = self.nc
    probe = nc.sync.nop(nofuse=True, hint="tail_waits")
    wait_clock.add_sem_waits(probe.ins, ScopedClock({None: tick_clock.global_clock}))
    waits = list(probe.ins.sync_info.on_wait or [])
    if len(waits) > 1:
        probe.ins.sync_info.on_wait = waits[:1]
        for i in range(1, len(waits)):
            extra = nc.sync.nop(nofuse=True, hint=f"tail_waits_{i}")
            si = extra.ins.sync_info
            if si is None:
                extra.ins.sync_info = mybir.SyncInfo(on_wait=[waits[i]], on_update=[])
            else:
                si.on_wait = [waits[i]]
    nc.sync.drain()
    nc.all_engine_barrier()
    popped = nc._tile_sem_poison_stack.pop()
    assert popped is self._sem_poison
    nc.clear_and_free_semaphores(list(self.sems.allocated().values()))
    nc.all_engine_barrier()


tile.TileContext._drain_and_barrier = _patched_drain_and_barrier

# Same walrus restriction for regular instructions: spill extra sem waits
# onto preceding same-engine NOPs at commit time.
_orig_commit = tile.TileContext._commit_instruction


def _patched_commit(self, inst, *args, **kwargs):
    si = getattr(inst, 'sync_info', None)
    if si is not None and si.on_wait and len(si.on_wait) > 1 \
            and inst.engine != mybir.EngineType.Unassigned:
        waits = list(si.on_wait)
        si.on_wait = waits[-1:]
        for w in waits[:-1]:
            nop = mybir.InstNoOp(
                name=self.nc.get_next_instruction_name(),
                ins=[], outs=[], bass_is_fusable=False)
            nop.engine = inst.engine
            nop.sync_info = mybir.SyncInfo(on_wait=[w], on_update=[])
            _orig_commit(self, nop, *args, **kwargs)
    return _orig_commit(self, inst, *args, **kwargs)


tile.TileContext._commit_instruction = _patched_commit


# ------------------------------------------------------------- device code
def _build_bass():
    nc = bass.Bass('TRN2', target_bir_lowering=False, debug=False)

    xin = nc.dram_tensor('xin', [T, KP, NCOL], FP8, kind='ExternalInput')
    smat_d = nc.dram_tensor('smat', [KP, 3 * M], FP8, kind='ExternalInput')
    cbias_d = nc.dram_tensor('cbias', [M, 3], FP32, kind='ExternalInput')
    wx_d = nc.dram_tensor('wx', [M, 3 * 96], BF16, kind='ExternalInput')
    wc_d = nc.dram_tensor('wc', [U + 1, 96], BF16, kind='ExternalInput')
    outh_d = nc.dram_tensor('outh', [U, 1], FP32, kind='ExternalOutput')

    Relu = mybir.ActivationFunctionType.Relu
    Sigmoid = mybir.ActivationFunctionType.Sigmoid
    Tanh = mybir.ActivationFunctionType.Tanh
    Copy = mybir.ActivationFunctionType.Copy
    Amax = mybir.AluOpType.max
    Aadd = mybir.AluOpType.add
    Asub = mybir.AluOpType.subtract
    Amul = mybir.AluOpType.mult
    DR = mybir.MatmulPerfMode.DoubleRow

    with tile.TileContext(nc) as tc:
        const = tc.alloc_tile_pool(name='const', bufs=1)
        state = tc.alloc_tile_pool(name='state', bufs=3)
        stackp = tc.alloc_tile_pool(name='stack', bufs=4)
        psum = tc.alloc_tile_pool(name='psum', bufs=2, space='PSUM')
        gs = tc.alloc_tile_pool(name='gs', bufs=5)
        ga_pool = tc.alloc_tile_pool(name='ga', bufs=3)
        tmp = tc.alloc_tile_pool(name='tmp', bufs=4)

        # constants
        smat = const.tile([KP, 3 * M], FP8, tag='sm')
        nc.sync.dma_start(smat[:], smat_d[:])
        cbias = const.tile([M, 3], FP32, tag='cb')
        nc.sync.dma_start(cbias[:], cbias_d[:])
        wx = const.tile([M, 3, 96], BF16, tag='wx')
        nc.sync.dma_start(wx[:], wx_d[:].rearrange("p (g n) -> p g n", g=3))
        wc = const.tile([U + 1, 96], BF16, tag='wc')
        nc.sync.dma_start(wc[:], wc_d[:])
        zer = const.tile([M, NCOL], FP32, tag='zer')
        nc.vector.memset(zer[:], 0.0)
        zv = zer[:].rearrange("p (u q) -> p u q", u=4)

        # scan state: cell [33,1] (row 32 = const 1.0 feeds the bias row of
        # wc), hidden [32,1]; manual 3-deep rotation of persistent tiles.
        cbufs = [state.tile([U + 1, 1], BF16, tag=f'c{i}', name=f'cbuf{i}')
                 for i in range(3)]
        hbufs = [state.tile([U, 1], BF16, tag=f'h{i}', name=f'hbuf{i}')
                 for i in range(3)]
        for cb in cbufs:
            nc.vector.memset(cb[:], 0.0)
            nc.vector.memset(cb[U:U + 1, :], 1.0)
        for hb in hbufs:
            nc.vector.memset(hb[:], 0.0)
        cprev, hprev = cbufs[2], hbufs[2]

        stacks = [None] * T
        gsums = [None] * T

        def get_stack(t):
            if stacks[t] is None:
                st = stackp.tile([KP, NCOL], FP8, tag='stk')
                nc.sync.dma_start(st[:], xin[t])
                stacks[t] = st
            return stacks[t]

        def emit_conv(t, scan_t):
            st = get_stack(t)
            if t + 2 < T:
                get_stack(t + 2)  # prefetch
            gsA = gs.tile([M, 3], BF16, tag='gsA')
            gsB = gs.tile([M, 3], BF16, tag='gsB')
            gsum = gs.tile([M, 3], BF16, tag='gsum')
            gsums[t] = gsum
            for g in range(3):
                pt = psum.tile([M, 2048], FP32, tag='pt')
                if g == 0 and scan_t is not None:
                    # scan step scan_t first: gates matmuls into this tile's
                    # slack column + the sigmoid/ALU chain land at each
                    # engine queue's head with frame-old deps, executing
                    # under frame t's conv work.
                    emit_scan(scan_t, pt)
                for k in range(4):
                    nc.tensor.matmul(
                        pt[:, 512 * k:512 * k + NCH],
                        smat[:, M * g:M * (g + 1)],
                        st[:, NCH * k:NCH * (k + 1)],
                        start=True, stop=True,
                        tile_position=(0, 0))
                # split the ReLU+GAP across both engines: Scalar takes the
                # first SBK banks, Vector the rest (tile drains faster than
                # PE refills, keeping PE busy for the HAM clock gate)
                ptv = pt.rearrange("p (u q) -> p u q", q=512)[:, :, 0:NCH]
                with nc.allow_low_precision(reason="GAP partials in bf16"):
                    nc.scalar.activation(ptv[:, 0:SBK, :], ptv[:, 0:SBK, :],
                                         Relu, bias=cbias[:, g:g + 1],
                                         accum_out=gsA[:, g:g + 1])
                    nc.vector.scalar_tensor_tensor(
                        out=ptv[:, SBK:4, :], in0=ptv[:, SBK:4, :],
                        scalar=cbias[:, g:g + 1], in1=zv[:, SBK:4, :],
                        op0=Aadd, op1=Amax,
                        accum_out=gsB[:, g:g + 1])
            with nc.allow_low_precision(reason="GAP partials in bf16"):
                nc.gpsimd.tensor_add(gsum[:], gsA[:], gsB[:])

        def emit_scan(t, slot):
            nonlocal cprev, hprev
            gsum = gsums[t]
            gp = slot[0:96, 1960:1961]
            for g in range(3):
                nc.tensor.matmul(gp, wx[:, g, :], gsum[:, g:g + 1],
                                 start=(g == 0), stop=False,
                                 tile_position=(0, 0), skip_group_check=True)
            nc.tensor.matmul(gp, wc[:], cprev[:], start=False, stop=True,
                             tile_position=(0, 0), skip_group_check=True)
            gsums[t] = None

            # gate rows 64:96 hold 2*(a3+b3) (x2 folded into stationaries):
            # tanh(a3) = 2*sigmoid(2*a3+2*b3) - 1. All cross-gate products
            # are staged at partition base 32 (walrus requires equal operand
            # bases); single-src ACT outputs may shift partitions freely.
            # fp32 gate intermediates; bf16 only for stored c/h
            sab = ga_pool.tile([2 * U, 1], FP32, tag='sab')
            nc.scalar.activation(sab[:], gp[0:2 * U, :], Sigmoid)
            s3t = ga_pool.tile([2 * U, 1], FP32, tag='s3t')
            nc.scalar.activation(s3t[U:2 * U, :], gp[2 * U:3 * U, :], Tanh)

            t1v = tmp.tile([2 * U, 1], FP32, tag='t1')
            nc.gpsimd.tensor_mul(t1v[U:2 * U, :], sab[0:U, :], hprev[:])
            t2v = tmp.tile([2 * U, 1], FP32, tag='t2')
            nc.gpsimd.tensor_mul(t2v[U:2 * U, :], sab[U:2 * U, :],
                                 s3t[U:2 * U, :])
            cnew = cbufs[t % 3]
            nc.vector.tensor_add(cnew[0:U, :], t1v[U:2 * U, :],
                                 t2v[U:2 * U, :])
            th = tmp.tile([U, 1], FP32, tag='th')
            nc.scalar.activation(th[:], cnew[0:U, :], Tanh)
            hnew = hbufs[t % 3]
            nc.vector.tensor_mul(hnew[:], th[:], cnew[0:U, :])
            cprev, hprev = cnew, hnew

        LAG = 2  # scan step t-LAG emitted at the head of frame t
        for t in range(T):
            emit_conv(t, t - LAG if t >= LAG else None)
        for t in range(T - LAG, T):
            tail = psum.tile([M, 2048], FP32, tag='pt')
            emit_scan(t, tail)

        hout = tmp.tile([U, 1], FP32, tag='hout')
        nc.scalar.activation(hout[:], hprev[:], Copy)
        nc.sync.dma_start(outh_d[:], hout[:])

        for p in (tmp, ga_pool, gs, psum, stackp, state, const):
            p.release()

    return nc


# -------------------------------------------------------------- host prep
def _prep_inputs(x, conv_w, conv_b, W1, b1, W2, b2, W3, b3):
    x = np.asarray(x, np.float32)
    conv_w = np.asarray(conv_w, np.float32)
    conv_b = np.asarray(conv_b, np.float32)

    # --- fp8 stack: [B, T, 45, 2, 1568], tap tau = c*30 + dx*10 + wr ---
    xp = np.zeros((B, T, H + 2, W + 2, C), np.float32)
    xp[:, :, 1:H + 1, 1:W + 1, :] = x
    stack = np.empty((B, T, 90, JA, W), np.float32)
    for c in range(C):
        for dx in range(3):
            for wr in range(10):
                tau = c * 30 + dx * 10 + wr
                stack[:, :, tau] = xp[:, :, wr:wr + 8 * (JA - 1) + 1:8,
                                      dx:dx + W, c]
    xin = stack.reshape(B, T, KP, NCOL).astype(ml_dtypes.float8_e4m3fn)

    # --- fp8 stationaries: [45, 2, 3*128], col j = f_loc*8 + i ---
    smat = np.zeros((90, 3, M), np.float32)
    for c in range(C):
        for dx in range(3):
            for wr in range(10):
                tau = c * 30 + dx * 10 + wr
                for i in range(8):
                    dy = wr - i
                    if 0 <= dy <= 2:
                        for g in range(3):
                            fl = np.arange(16)
                            smat[tau, g, fl * 8 + i] = conv_w[dy, dx, c,
                                                              g * 16 + fl]
    smat = smat.reshape(KP, 3 * M).astype(ml_dtypes.float8_e4m3fn)

    cbias = np.empty((M, 3), np.float32)
    for g in range(3):
        cbias[:, g] = np.repeat(conv_b[g * 16:(g + 1) * 16], 8)

    # --- scan weights ---
    Wall = np.stack([np.asarray(Wg, np.float32) for Wg in (W1, W2, W3)], axis=1)
    Wall = Wall.reshape(F + U, 96)  # rows: feats 0..47, hidden 48..79
    wxf = Wall[0:F] / HWN           # [48, 96]
    wx = np.empty((M, 3 * 96), np.float32)
    for g in range(3):
        # partition j = f_loc*8 + i replicates the filter row across i
        wx[:, 96 * g:96 * (g + 1)] = np.repeat(wxf[g * 16:(g + 1) * 16],
                                               8, axis=0)
    gb = np.concatenate([np.asarray(b, np.float32) for b in (b1, b2, b3)])
    wc = np.concatenate([Wall[F:F + U], gb.reshape(1, 96)], axis=0)  # [33,96]

    return (xin, smat, cbias,
            wx.astype(ml_dtypes.bfloat16), wc.astype(ml_dtypes.bfloat16))


# ------------------------------------------------------------------ kernel
def kernel(x, conv_w, conv_b, W1, b1, W2, b2, W3, b3, W4, b4):
    global LAST_RESULTS
    xin, smat, cbias, wx, wc = _prep_inputs(
        x, conv_w, conv_b, W1, b1, W2, b2, W3, b3)

    nc = _build_bass()
    in_maps = [{
        'xin': np.ascontiguousarray(xin[b]),
        'smat': smat,
        'cbias': cbias,
        'wx': wx,
        'wc': wc,
    } for b in range(B)]

    res = run_bass_kernel_spmd(nc, in_maps, core_ids=list(range(B)))
    LAST_RESULTS = res
    out = np.stack([res.results[b]['outh'][:, 0] for b in range(B)], axis=0)
    return out.astype(np.float32)


# revision 15
# speedup vs baseline: 1.1881x; 1.1881x over previous
"""Trainium2 Bass kernel for BasicCNN+LSTM (conv3x3+ReLU+GAP -> custom LSTM scan).

Self-contained: hardcodes shapes/sharding. Data-parallel over batch B=8 across
8 NeuronCores; each core processes one batch element end-to-end, the host
gathers the 8 [32] results.

v2 design (vs. baseline's 2-pixel bf16 scheme):
  - Conv as fp8(e4m3) DoubleRow matmuls: 8-pixel vertical blocks, K=90 taps
    (3c x 3dx x 10 window rows) split into 2 k-tiles of 45, M=128 = 16
    filters x 8 pixel positions (f-major), 3 filter groups. Moving columns
    per frame: 3 x 1568 (25% less than baseline, at 2x fp8 column rate),
    robust to the PE HAM clock gate (cold 1.2GHz still fits).
  - ReLU(+bias)+GAP as ONE instruction per (frame, filter-group) over a
    strided [128, 4, 392] view of a 4-bank PSUM tile, alternating
    Scalar ACT / Vector tensor_scalar, each with a bf16 accum_out column.
  - Scan step: 4 tiny accumulating matmuls (3x K=128 reading the gsum
    columns directly - the stationary replicates weights across the 8 pixel
    positions so the cross-partition fold is absorbed - plus K=32 for the
    cell part), one sigmoid ACT with per-partition scale [1,1,2] and bias
    [b1,b2,2*b3] (tanh(a)=2*sigmoid(2a)-1), and 6 tiny ALU ops spread over
    GpSimd/Vector. Reference's state-order swap bug kept: the z "hidden"
    input is the previous cell, and gate1 multiplies the previous hidden.
"""
import sys
if '/opt/trn_rl_repo' not in sys.path:
    sys.path.insert(0, '/opt/trn_rl_repo')

import numpy as np
import ml_dtypes

import concourse.bass as bass
import concourse.mybir as mybir
import concourse.tile as tile
from concourse.vector_clock import ScopedClock
from concourse.bass_utils import run_bass_kernel_spmd

# ---------------------------------------------------------------- constants
B, T, H, W, C, F, U = 8, 24, 112, 112, 3, 48, 32
KP = 90            # taps: 3c x 3dx x 10 window rows
JA = 14            # vertical 8-pixel blocks per frame (112/8)
NCOL = JA * W      # 1568 moving columns per frame per filter group
NCH = 392          # columns per matmul chunk (4 chunks, bank-aligned @512)
M = 128            # out partitions = 16 filters x 8 pixel positions
SBK = 2            # banks of each psum tile handled by ScalarE (rest: DVE)
HWN = float(H * W)

FP32 = mybir.dt.float32
BF16 = mybir.dt.bfloat16
FP8 = mybir.dt.float8e4

LAST_RESULTS = None  # BassKernelResults of the most recent run (for test.py)

# ------------------------------------------------- TileContext drain patch
# The container's walrus rejects >1 semaphore wait per instruction; Tile's
# kernel-tail drain aggregates all end-of-kernel waits onto one Drain.
# Spread them across single-wait NOPs on the sync engine instead.
def _patched_drain_and_barrier(self, tick_clock, wait_clock):
    nc = self.nc
    probe = nc.sync.nop(nofuse=True, hint="tail_waits")
    wait_clock.add_sem_waits(probe.ins, ScopedClock({None: tick_clock.global_clock}))
    waits = list(probe.ins.sync_info.on_wait or [])
    if len(waits) > 1:
        probe.ins.sync_info.on_wait = waits[:1]
        for i in range(1, len(waits)):
            extra = nc.sync.nop(nofuse=True, hint=f"tail_waits_{i}")
            si = extra.ins.sync_info
            if si is None:
                extra.ins.sync_info = mybir.SyncInfo(on_wait=[waits[i]], on_update=[])
            else:
                si.on_wait = [waits[i]]
    nc.sync.drain()
    nc.all_engine_barrier()
    popped = nc._tile_sem_poison_stack.pop()
    assert popped is self._sem_poison
    nc.clear_and_free_semaphores(list(self.sems.allocated().values()))
    nc.all_engine_barrier()


tile.TileContext._drain_and_barrier = _patched_drain_and_barrier

# Same walrus restriction for regular instructions: spill extra sem waits
# onto preceding same-engine NOPs at commit time.
_orig_commit = tile.TileContext._commit_instruction


def _patched_commit(self, inst, *args, **kwargs):
    si = getattr(inst, 'sync_info', None)
    if si is not None and si.on_wait and len(si.on_wait) > 1 \
            and inst.engine != mybir.EngineType.Unassigned:
        waits = list(si.on_wait)
        si.on_wait = waits[-1:]
        for w in waits[:-1]:
            nop = mybir.InstNoOp(
                name=self.nc.get_next_instruction_name(),
                ins=[], outs=[], bass_is_fusable=False)
            nop.engine = inst.engine
            nop.sync_info = mybir.SyncInfo(on_wait=[w], on_update=[])
            _orig_commit(self, nop, *args, **kwargs)
    return _orig_commit(self, inst, *args, **kwargs)


tile.TileContext._commit_instruction = _patched_commit


# ------------------------------------------------------------- device code
def _build_bass():
    nc = bass.Bass('TRN2', target_bir_lowering=False, debug=False)

    xin = nc.dram_tensor('xin', [T, KP, NCOL], FP8, kind='ExternalInput')
    smat_d = nc.dram_tensor('smat', [KP, 3 * M], FP8, kind='ExternalInput')
    cbias_d = nc.dram_tensor('cbias', [M, 3], FP32, kind='ExternalInput')
    wx_d = nc.dram_tensor('wx', [M, 3 * 96], BF16, kind='ExternalInput')
    wc_d = nc.dram_tensor('wc', [U + 1, 96], BF16, kind='ExternalInput')
    outh_d = nc.dram_tensor('outh', [U, 1], FP32, kind='ExternalOutput')

    Relu = mybir.ActivationFunctionType.Relu
    Sigmoid = mybir.ActivationFunctionType.Sigmoid
    Tanh = mybir.ActivationFunctionType.Tanh
    Copy = mybir.ActivationFunctionType.Copy
    Amax = mybir.AluOpType.max
    Aadd = mybir.AluOpType.add
    Asub = mybir.AluOpType.subtract
    Amul = mybir.AluOpType.mult
    DR = mybir.MatmulPerfMode.DoubleRow

    with tile.TileContext(nc) as tc:
        const = tc.alloc_tile_pool(name='const', bufs=1)
        state = tc.alloc_tile_pool(name='state', bufs=3)
        stackp = tc.alloc_tile_pool(name='stack', bufs=4)
        psum = tc.alloc_tile_pool(name='psum', bufs=2, space='PSUM')
        gs = tc.alloc_tile_pool(name='gs', bufs=5)
        ga_pool = tc.alloc_tile_pool(name='ga', bufs=3)
        tmp = tc.alloc_tile_pool(name='tmp', bufs=4)

        # constants
        smat = const.tile([KP, 3 * M], FP8, tag='sm')
        nc.sync.dma_start(smat[:], smat_d[:])
        cbias = const.tile([M, 3], FP32, tag='cb')
        nc.sync.dma_start(cbias[:], cbias_d[:])
        wx = const.tile([M, 3, 96], BF16, tag='wx')
        nc.sync.dma_start(wx[:], wx_d[:].rearrange("p (g n) -> p g n", g=3))
        wc = const.tile([U + 1, 96], BF16, tag='wc')
        nc.sync.dma_start(wc[:], wc_d[:])
        zer = const.tile([M, NCOL], FP32, tag='zer')
        nc.vector.memset(zer[:], 0.0)
        zv = zer[:].rearrange("p (u q) -> p u q", u=4)

        # scan state: cell [33,1] (row 32 = const 1.0 feeds the bias row of
        # wc), hidden [32,1]; manual 3-deep rotation of persistent tiles.
        cbufs = [state.tile([U + 1, 1], BF16, tag=f'c{i}', name=f'cbuf{i}')
                 for i in range(3)]
        hbufs = [state.tile([U, 1], BF16, tag=f'h{i}', name=f'hbuf{i}')
                 for i in range(3)]
        for cb in cbufs:
            nc.vector.memset(cb[:], 0.0)
            nc.vector.memset(cb[U:U + 1, :], 1.0)
        for hb in hbufs:
            nc.vector.memset(hb[:], 0.0)
        cprevs = {-1: cbufs[2][:]}
        for i in range(T):
            cprevs[i] = cbufs[i % 3][:]

        stacks = [None] * T
        gsums = [None] * T

        def get_stack(t):
            if stacks[t] is None:
                st = stackp.tile([KP, NCOL], FP8, tag='stk')
                nc.sync.dma_start(st[:], xin[t])
                stacks[t] = st
            return stacks[t]

        def emit_conv(t):
            st = get_stack(t)
            if t + 2 < T:
                get_stack(t + 2)  # prefetch
            gsA = gs.tile([M, 3], BF16, tag='gsA')
            gsB = gs.tile([M, 3], BF16, tag='gsB')
            gsum = gs.tile([M, 3], BF16, tag='gsum')
            gsums[t] = gsum
            pt = None
            for g in range(3):
                pt = psum.tile([M, 2048], FP32, tag='pt')
                for k in range(4):
                    nc.tensor.matmul(
                        pt[:, 512 * k:512 * k + NCH],
                        smat[:, M * g:M * (g + 1)],
                        st[:, NCH * k:NCH * (k + 1)],
                        start=True, stop=True,
                        tile_position=(0, 0))
                # split the ReLU+GAP across both engines: Scalar takes the
                # first SBK banks, Vector the rest
                ptv = pt.rearrange("p (u q) -> p u q", q=512)[:, :, 0:NCH]
                with nc.allow_low_precision(reason="GAP partials in bf16"):
                    nc.scalar.activation(ptv[:, 0:SBK, :], ptv[:, 0:SBK, :],
                                         Relu, bias=cbias[:, g:g + 1],
                                         accum_out=gsA[:, g:g + 1])
                    nc.vector.scalar_tensor_tensor(
                        out=ptv[:, SBK:4, :], in0=ptv[:, SBK:4, :],
                        scalar=cbias[:, g:g + 1], in1=zv[:, SBK:4, :],
                        op0=Aadd, op1=Amax,
                        accum_out=gsB[:, g:g + 1])
            with nc.allow_low_precision(reason="GAP partials in bf16"):
                nc.gpsimd.tensor_add(gsum[:], gsA[:], gsB[:])
            return pt

        def emit_gates(t, slot):
            # 4 accumulating matmuls into a slack column of `slot` (bank 3,
            # beyond the conv chunks; has_written is per-element so the
            # conv chunk in the same bank is unaffected)
            gsum = gsums[t]
            gp = slot[0:96, 1960:1961]
            for g in range(3):
                nc.tensor.matmul(gp, wx[:, g, :], gsum[:, g:g + 1],
                                 start=(g == 0), stop=False,
                                 tile_position=(0, 0), skip_group_check=True)
            nc.tensor.matmul(gp, wc[:], cprevs[t - 1], start=False, stop=True,
                             tile_position=(0, 0), skip_group_check=True)
            gsums[t] = None

        def emit_chain(t, slot):
            gp = slot[0:96, 1960:1961]
            # fp32 gate intermediates; bf16 only for stored c/h
            sab = ga_pool.tile([2 * U, 1], FP32, tag='sab')
            nc.scalar.activation(sab[:], gp[0:2 * U, :], Sigmoid)
            s3t = ga_pool.tile([2 * U, 1], FP32, tag='s3t')
            nc.scalar.activation(s3t[U:2 * U, :], gp[2 * U:3 * U, :], Tanh)

            t1v = tmp.tile([2 * U, 1], FP32, tag='t1')
            nc.gpsimd.tensor_mul(t1v[U:2 * U, :], sab[0:U, :],
                                 hbufs[(t - 1) % 3][:])
            t2v = tmp.tile([2 * U, 1], FP32, tag='t2')
            nc.gpsimd.tensor_mul(t2v[U:2 * U, :], sab[U:2 * U, :],
                                 s3t[U:2 * U, :])
            cnew = cbufs[t % 3]
            nc.vector.tensor_add(cnew[0:U, :], t1v[U:2 * U, :],
                                 t2v[U:2 * U, :])
            th = tmp.tile([U, 1], FP32, tag='th')
            nc.scalar.activation(th[:], cnew[0:U, :], Tanh)
            hnew = hbufs[t % 3]
            nc.vector.tensor_mul(hnew[:], th[:], cnew[0:U, :])

        # Scan step s: gates matmuls at the END of frame s+LAG-1 (PE deps
        # are then a frame old), sigmoid/ALU chain at the HEAD of frame
        # s+LAG (executes under that frame's conv, no FIFO head-of-line
        # blocking of the ReLU+GAP work on any queue).
        LAG = 3
        gate_slot = None
        for t in range(T):
            if t >= LAG:
                emit_chain(t - LAG, gate_slot)
            pt_last = emit_conv(t)
            s = t - LAG + 1
            if 0 <= s < T:
                emit_gates(s, pt_last)
                gate_slot = pt_last
        for s in range(T - LAG, T):
            emit_chain(s, gate_slot)
            if s + 1 < T:
                tail = psum.tile([M, 2048], FP32, tag='pt')
                emit_gates(s + 1, tail)
                gate_slot = tail

        hout = tmp.tile([U, 1], FP32, tag='hout')
        nc.scalar.activation(hout[:], hbufs[(T - 1) % 3][:], Copy)
        nc.sync.dma_start(outh_d[:], hout[:])

        for p in (tmp, ga_pool, gs, psum, stackp, state, const):
            p.release()

    return nc


# -------------------------------------------------------------- host prep
def _prep_inputs(x, conv_w, conv_b, W1, b1, W2, b2, W3, b3):
    x = np.asarray(x, np.float32)
    conv_w = np.asarray(conv_w, np.float32)
    conv_b = np.asarray(conv_b, np.float32)

    # --- fp8 stack: [B, T, 45, 2, 1568], tap tau = c*30 + dx*10 + wr ---
    xp = np.zeros((B, T, H + 2, W + 2, C), np.float32)
    xp[:, :, 1:H + 1, 1:W + 1, :] = x
    stack = np.empty((B, T, 90, JA, W), np.float32)
    for c in range(C):
        for dx in range(3):
            for wr in range(10):
                tau = c * 30 + dx * 10 + wr
                stack[:, :, tau] = xp[:, :, wr:wr + 8 * (JA - 1) + 1:8,
                                      dx:dx + W, c]
    xin = stack.reshape(B, T, KP, NCOL).astype(ml_dtypes.float8_e4m3fn)

    # --- fp8 stationaries: [45, 2, 3*128], col j = f_loc*8 + i ---
    smat = np.zeros((90, 3, M), np.float32)
    for c in range(C):
        for dx in range(3):
            for wr in range(10):
                tau = c * 30 + dx * 10 + wr
                for i in range(8):
                    dy = wr - i
                    if 0 <= dy <= 2:
                        for g in range(3):
                            fl = np.arange(16)
                            smat[tau, g, fl * 8 + i] = conv_w[dy, dx, c,
                                                              g * 16 + fl]
    smat = smat.reshape(KP, 3 * M).astype(ml_dtypes.float8_e4m3fn)

    cbias = np.empty((M, 3), np.float32)
    for g in range(3):
        cbias[:, g] = np.repeat(conv_b[g * 16:(g + 1) * 16], 8)

    # --- scan weights ---
    Wall = np.stack([np.asarray(Wg, np.float32) for Wg in (W1, W2, W3)], axis=1)
    Wall = Wall.reshape(F + U, 96)  # rows: feats 0..47, hidden 48..79
    wxf = Wall[0:F] / HWN           # [48, 96]
    wx = np.empty((M, 3 * 96), np.float32)
    for g in range(3):
        # partition j = f_loc*8 + i replicates the filter row across i
        wx[:, 96 * g:96 * (g + 1)] = np.repeat(wxf[g * 16:(g + 1) * 16],
                                               8, axis=0)
    gb = np.concatenate([np.asarray(b, np.float32) for b in (b1, b2, b3)])
    wc = np.concatenate([Wall[F:F + U], gb.reshape(1, 96)], axis=0)  # [33,96]

    return (xin, smat, cbias,
            wx.astype(ml_dtypes.bfloat16), wc.astype(ml_dtypes.bfloat16))


# ------------------------------------------------------------------ kernel
def kernel(x, conv_w, conv_b, W1, b1, W2, b2, W3, b3, W4, b4):
    global LAST_RESULTS
    xin, smat, cbias, wx, wc = _prep_inputs(
        x, conv_w, conv_b, W1, b1, W2, b2, W3, b3)

    nc = _build_bass()
    in_maps = [{
        'xin': np.ascontiguousarray(xin[b]),
        'smat': smat,
        'cbias': cbias,
        'wx': wx,
        'wc': wc,
    } for b in range(B)]

    res = run_bass_kernel_spmd(nc, in_maps, core_ids=list(range(B)))
    LAST_RESULTS = res
    out = np.stack([res.results[b]['outh'][:, 0] for b in range(B)], axis=0)
    return out.astype(np.float32)
